# revision 1
# baseline (speedup 1.0000x reference)
"""Trainium2 Bass kernel for a Neural CDE (Euler scan over cubic-spline control).

Strategy
--------
Data-parallel over batch: B=256 -> 8 cores x 32. The T=512 Euler steps are
inherently sequential (y_{k+1} depends on y_k), so each core runs the full
scan on its batch shard with all MLP weights resident in SBUF.

Per step the dominant op is f = tanh(g2 @ f_w2.T + b2) with f_w2 [8192, 256].
We run it weights-stationary on the PE (fp16 weights -> fast-weight-load),
emitting f TRANSPOSED as 64 chunks [128(h), 32(b)], one chunk per spline
dim d (f_w2 rows are pre-permuted d-major on the host). That layout gives
full 128-partition utilization for the tanh (ACT) and lets the einsum
  y += sum_d f[:,h,d] * deriv[:,d]
become one contiguous packed-fp16 DVE multiply by a partition-broadcast
deriv row plus a strided segmented reduce.

The cubic-spline derivative deriv_k is state-independent, so it is
precomputed on the host (exactly mirroring the reference fp32 arithmetic),
pre-transposed d-major in fp16, and replicated across partitions by the
(otherwise idle) DMA engines via a stride-0 source read, ahead of use.
"""

import os
import sys

import numpy as np

for _p in ("/opt/trn_rl_repo",):
    if _p not in sys.path:
        sys.path.insert(0, _p)

import ml_dtypes  # noqa: E402

import concourse.bass as bass  # noqa: E402
import concourse.tile as tile  # noqa: E402
from concourse import library_config, mybir  # noqa: E402
from concourse.bass_utils import run_bass_kernel_spmd  # noqa: E402

B, T, D, H, W = 256, 512, 64, 128, 256
N_CORES = 8
BL = B // N_CORES  # 32 batch per core
HD = H * D  # 8192
NJ = D  # 64 output chunks of [128, 32], one per spline dim d
N_BANKS = 4  # PSUM banks holding f^T: 16 chunks x [128,32] each
JPB = NJ // N_BANKS  # 16 chunks per bank

F32 = mybir.dt.float32
F32R = mybir.dt.float32r
BF16 = mybir.dt.bfloat16
F16 = mybir.dt.float16
AFT = mybir.ActivationFunctionType

# Matmul dtype for the three per-step MLP matmuls (weights & activations).
# fp16: same PE/DVE/FWL speed as bf16 but 10 mantissa bits -- the weights
# (<=1/16), activations, tanh outputs and spline derivs are all well inside
# fp16 range, so quantization noise drops ~8x vs bf16.
MM_DTYPE = F16
# Steps actually emitted (patchable for fast simulation tests).
N_STEPS = int(os.environ.get("K_STEPS", T))
# deriv broadcast mechanism: dma (stride-0 partition-broadcast DMA) | gpsimd
BCAST = os.environ.get("K_BCAST", "dma")

# softplus(x) = relu(x) + log1p(exp(-|x|)); the walrus ACT tables here have
# no softplus set that coexists with tanh, so log1p is a minimax polynomial
# sum_{i=1..3} c_i e^i on e in [0,1], evaluated on the DVE via the
# (p + c_i) * e recurrence. Coefficients c3..c1 (max err ~1.1e-3 absolute,
# comparable to the fp16 rounding of g itself; each extra degree costs a
# serial ~220ns DVE op on the per-step head chain twice).
SP_C = [0.12855132, -0.42702028, 0.9926641]


def _f32(x):
    return np.ascontiguousarray(x, dtype=np.float32)


def _prep_host(inputs):
    """Host-side layout prep: shard batch, transpose weights into lhsT tile
    layouts, precompute the spline derivative for every step (exact fp32
    mirror of the reference's searchsorted/frac arithmetic)."""
    ts = np.asarray(inputs["ts"], np.float32)
    ca = np.asarray(inputs["coeffs_a"], np.float32)
    cb = np.asarray(inputs["coeffs_b"], np.float32)
    cc = np.asarray(inputs["coeffs_c"], np.float32)
    cd = np.asarray(inputs["coeffs_d"], np.float32)

    dt = np.float32(ts[1] - ts[0])
    # t carry exactly as the reference accumulates it (fp32 adds)
    t_seq = np.empty(T, np.float32)
    t_seq[0] = ts[0]
    for k in range(1, T):
        t_seq[k] = np.float32(t_seq[k - 1] + dt)
    idx = np.clip(np.searchsorted(ts, t_seq, side="right") - 1, 0, T - 2)
    frac = (t_seq - ts[idx]).astype(np.float32)
    # deriv[k] = b[:,idx] + 2*c[:,idx]*frac + 3*d[:,idx]*frac^2   [B, T, D]
    f1 = frac[None, :, None]
    deriv = cb[:, idx, :] + np.float32(2.0) * cc[:, idx, :] * f1 \
        + np.float32(3.0) * cd[:, idx, :] * (f1 * f1)
    deriv = _f32(deriv * dt)  # fold the Euler dt (dt==1.0 -> exact)

    w0 = np.asarray(inputs["f_w0"], np.float32)  # [W=256, H=128]
    w1 = np.asarray(inputs["f_w1"], np.float32)  # [256, 256]
    w2 = np.asarray(inputs["f_w2"], np.float32)  # [8192, 256]
    b0 = np.asarray(inputs["f_b0"], np.float32)
    b1 = np.asarray(inputs["f_b1"], np.float32)
    b2 = np.asarray(inputs["f_b2"], np.float32)
    iw0 = np.asarray(inputs["i_w0"], np.float32)  # [256, 64]
    iw1 = np.asarray(inputs["i_w1"], np.float32)  # [256, 256]
    iw2 = np.asarray(inputs["i_w2"], np.float32)  # [128, 256]
    ib0 = np.asarray(inputs["i_b0"], np.float32)
    ib1 = np.asarray(inputs["i_b1"], np.float32)
    ib2 = np.asarray(inputs["i_b2"], np.float32)
    lw = np.asarray(inputs["l_w"], np.float32)  # [1, 128]
    lb = np.asarray(inputs["l_b"], np.float32)  # [1]

    mmnp = (np.float16 if MM_DTYPE == F16 else
            ml_dtypes.bfloat16 if MM_DTYPE == BF16 else np.float32)

    shared = {
        # func MLP, lhsT tile layouts ([K, M] per 128-col group)
        "w0": np.ascontiguousarray(w0.T, dtype=mmnp),  # [128, 256]
        "w1": np.ascontiguousarray(
            w1.reshape(2, 128, 2, 128).transpose(3, 2, 0, 1).reshape(128, 512),
            dtype=mmnp),
        # d-major chunk permutation: tile (kappa, j) cols -> f_w2[m*64+j, kappa*128+k]
        "w2": np.ascontiguousarray(
            w2.reshape(128, 64, 2, 128).transpose(3, 2, 1, 0).reshape(128, 16384),
            dtype=mmnp),
        "b0": _f32(b0.reshape(2, 128).T),  # [128, 2]
        "b1": _f32(b1.reshape(2, 128).T),
        # b2 added in PSUM by the PE via a one-hot matmul: for bank q,
        # psum[m, j16*32+b] += sum_j' b2mm[j', q*128+m] * onehot[j', j16*32+b]
        "b2mm": np.ascontiguousarray(
            b2.reshape(128, 4, 16).transpose(2, 1, 0).reshape(16, 512), dtype=mmnp),
        "onehot": np.ascontiguousarray(
            np.repeat(np.eye(16, dtype=np.float32), 32, axis=1), dtype=mmnp),
        # initial MLP (run once, f32r)
        "iw0": _f32(iw0.T),  # [64, 256]
        "iw1": _f32(iw1.reshape(2, 128, 2, 128).transpose(3, 2, 0, 1).reshape(128, 512)),
        "iw2": _f32(iw2.reshape(128, 2, 128).transpose(2, 1, 0).reshape(128, 256)),
        "ib0": _f32(ib0.reshape(2, 128).T),
        "ib1": _f32(ib1.reshape(2, 128).T),
        "ib2": _f32(ib2.reshape(128, 1)),
        "lw": _f32(lw.reshape(128, 1)),
        # pre-halved: sigmoid(z) = 0.5 + 0.5*tanh(0.5*z + 0.5*l_b)
        "lb": _f32(lb.reshape(1, 1) * 0.5),
    }

    per_core = []
    for s in range(N_CORES):
        sl = slice(s * BL, (s + 1) * BL)
        m = dict(shared)
        # x0 = coeffs_a[:, 0, :] transposed -> [64, 32]
        m["x0t"] = _f32(ca[sl, 0, :].T)
        # deriv per step, d-major (matching the f^T chunk layout) in fp16:
        # one [1, 2048] row per step; broadcast to 128 partitions off the DVE.
        m["derivt"] = np.ascontiguousarray(
            deriv[sl].transpose(1, 2, 0).reshape(T, D * BL), dtype=np.float16)
        per_core.append(m)
    return per_core


def _legalize_waits(nc, max_waits=1):
    """This walrus build allows only one embedded sem-wait on most
    instruction encodings. Spill extra waits onto standalone same-engine
    EventSemaphore instructions placed immediately before."""
    n_spilled = 0
    for f in nc.m.functions:
        for blk in f.blocks:
            out = []
            for inst in blk.instructions:
                si = inst.sync_info
                if si is not None and len(si.on_wait) > max_waits:
                    waits = list(si.on_wait)
                    for j, w in enumerate(waits[:-max_waits]):
                        ev = mybir.InstEventSemaphore(
                            name=f"{inst.name}-w{j}", engine=inst.engine,
                            ins=[], outs=[],
                            sync_info=mybir.SyncInfo(on_wait=[w], on_update=[]))
                        out.append(ev)
                        n_spilled += 1
                    inst.sync_info = mybir.SyncInfo(
                        on_wait=waits[-max_waits:], on_update=list(si.on_update))
                out.append(inst)
            blk.instructions = out
    return n_spilled


def build_program(n_steps=None, skip_b2=False, zero_b01=False):
    """Build the single-core Bass/Tile program (same program on all cores).

    skip_b2: omit the per-bank one-hot bias matmuls (valid when f_b2 == 0,
    which make_runner detects from the actual input data).
    zero_b01: f_b0 == f_b1 == 0 -> merged [128, 64] softplus head ops
    (ACT bias is per-partition, so halves with distinct biases can't merge)."""
    if n_steps is None:
        n_steps = N_STEPS
    nc = bass.Bass("TRN2", target_bir_lowering=False, debug=False,
                   enable_asserts=False, num_devices=N_CORES,
                   enable_partition_id=False)

    d_in = {}
    for name, shape, dtyp in [
        ("x0t", [D, BL], F32),
        ("derivt", [T, D * BL], F16),
        ("w0", [128, 256], MM_DTYPE),
        ("w1", [128, 512], MM_DTYPE),
        ("w2", [128, 16384], MM_DTYPE),
        ("b0", [128, 2], F32),
        ("b1", [128, 2], F32),
        ("b2mm", [16, 512], MM_DTYPE),
        ("onehot", [16, 512], MM_DTYPE),
        ("iw0", [64, 256], F32),
        ("iw1", [128, 512], F32),
        ("iw2", [128, 256], F32),
        ("ib0", [128, 2], F32),
        ("ib1", [128, 2], F32),
        ("ib2", [128, 1], F32),
        ("lw", [128, 1], F32),
        ("lb", [1, 1], F32),
    ]:
        d_in[name] = nc.dram_tensor(name, shape, dtyp, kind="ExternalInput").ap()
    out_dram = nc.dram_tensor("out", [1, BL], F32, kind="ExternalOutput").ap()

    with tile.TileContext(nc) as tc:
        from contextlib import ExitStack
        ctx = ExitStack()
        with ctx:
            cpool = ctx.enter_context(tc.tile_pool(name="const", bufs=1))
            psW = ctx.enter_context(tc.tile_pool(name="psW", bufs=1, space="PSUM"))
            psA = ctx.enter_context(tc.tile_pool(name="psA", bufs=3, space="PSUM"))
            psF = ctx.enter_context(tc.tile_pool(name="psF", bufs=4, space="PSUM"))
            dstage_pool = ctx.enter_context(tc.tile_pool(name="dstage", bufs=3))
            dbc_pool = ctx.enter_context(tc.tile_pool(name="dbc", bufs=4))
            ft_pool = ctx.enter_context(tc.tile_pool(name="ftp", bufs=2))
            act_pool = ctx.enter_context(tc.tile_pool(name="actp", bufs=2))
            pd_pool = ctx.enter_context(tc.tile_pool(name="pdp", bufs=8))

            if BCAST == "gpsimd":
                # partition_broadcast lives in the 'mlp' GPSIMD ucode library
                nc.gpsimd.load_library(library_config.mlp)

            # --- persistent SBUF tensors ---
            w0sb = cpool.tile_from(d_in["w0"], name="w0sb")
            w1sb = cpool.tile_from(d_in["w1"], name="w1sb")
            w2sb = cpool.tile_from(d_in["w2"], name="w2sb")
            b0sb = cpool.tile_from(d_in["b0"], name="b0sb")
            b1sb = cpool.tile_from(d_in["b1"], name="b1sb")
            b2sb = cpool.tile_from(d_in["b2mm"], name="b2sb")
            onehot = cpool.tile_from(d_in["onehot"], name="onehot")
            iw0sb = cpool.tile_from(d_in["iw0"], name="iw0sb")
            iw1sb = cpool.tile_from(d_in["iw1"], name="iw1sb")
            iw2sb = cpool.tile_from(d_in["iw2"], name="iw2sb")
            ib0sb = cpool.tile_from(d_in["ib0"], name="ib0sb")
            ib1sb = cpool.tile_from(d_in["ib1"], name="ib1sb")
            ib2sb = cpool.tile_from(d_in["ib2"], name="ib2sb")
            lwsb = cpool.tile_from(d_in["lw"], name="lwsb")
            lbsb = cpool.tile_from(d_in["lb"], name="lbsb")
            x0sb = cpool.tile_from(d_in["x0t"], name="x0sb")

            y32 = cpool.tile([128, BL], F32, name="y32")
            ybf = cpool.tile([128, BL], MM_DTYPE, name="ybf")

            # Warm each compute engine's vector clock over the const DMAs so
            # later instructions don't accumulate multiple sem waits (the
            # bias-carrying ACT encoding has room for only one).
            for wi, t in enumerate((b0sb, b1sb, ib0sb, ib1sb, ib2sb)):
                w = cpool.tile([128, 1], F32, name=f"warma{wi}")
                nc.scalar.copy(w, t[:, 0:1])
            warml = cpool.tile([1, 1], F32, name="warml")
            nc.scalar.copy(warml, lbsb[:, 0:1])
            for wi, t in enumerate((b0sb, b1sb, x0sb)):
                w = cpool.tile([t.shape[0], 1], F32, name=f"warmv{wi}")
                nc.vector.tensor_copy(w, t[:, 0:1])
            # PE: one junk self-pair matmul per weight DMA so real matmuls
            # never wait on more than one fresh semaphore.
            wjunk = psW.tile([128, 2], F32, name="wjunk")
            warm_mms = [x0sb, iw0sb, iw1sb, iw2sb, lwsb, w0sb, w1sb, b2sb, onehot]
            warm_mms += [w2sb[:, c:c + 1024] for c in range(0, 16384, 1024)]
            for t in warm_mms:
                mm = min(t.free_size(), 64)
                nc.tensor.matmul(wjunk[:mm, 0:1], t[:, 0:mm], t[:, 0:1],
                                 start=True, stop=True)

            # --- initial MLP: y0 = I2 @ relu(I1 @ relu(I0 @ x0)) (f32r) ---
            h0 = [None, None]
            for x in range(2):
                p = psA.tile([128, BL], F32, tag="ps_small", name="p_init0")
                nc.tensor.matmul(p, iw0sb[:, x * 128:(x + 1) * 128],
                                 x0sb[:, :], start=True, stop=True)
                h = act_pool.tile([128, BL], F32, tag="h_init", name="h_init0")
                nc.scalar.activation(h, p, AFT.Relu, bias=ib0sb[:, x:x + 1])
                h0[x] = h
            h1 = [None, None]
            for x in range(2):
                p = psA.tile([128, BL], F32, tag="ps_small", name="p_init1")
                for k in range(2):
                    nc.tensor.matmul(p, iw1sb[:, (k * 2 + x) * 128:(k * 2 + x + 1) * 128],
                                     h0[k][:, :], start=(k == 0), stop=(k == 1))
                h = act_pool.tile([128, BL], F32, tag="h_init2", name="h_init1")
                nc.scalar.activation(h, p, AFT.Relu, bias=ib1sb[:, x:x + 1])
                h1[x] = h
            p = psA.tile([128, BL], F32, tag="ps_small", name="p_init2")
            for k in range(2):
                nc.tensor.matmul(p, iw2sb[:, k * 128:(k + 1) * 128],
                                 h1[k][:, :], start=(k == 0), stop=(k == 1))
            nc.scalar.activation(y32, p, AFT.Identity, bias=ib2sb[:, 0:1])
            nc.scalar.copy(ybf, y32)

            # --- the scan ---
            for step in range(n_steps):
                # deriv row for this step: replicate the fp16 [1, 2048] DRAM
                # row to all 128 partitions in one DMA via a stride-0 repeat
                # on the source free axis. Reading the row 128x from HBM
                # parallelizes across banks/queues (a single-partition SBUF
                # source would serialize on one read port).
                dbc = dbc_pool.tile([128, D * BL], F16, tag="dbc", name="dbc")
                if BCAST == "dma":
                    row = d_in["derivt"][step:step + 1, :]
                    rep = bass.AP(row.tensor, row.offset,
                                  [list(row.ap[0]), [0, 128], [1, D * BL]])
                    nc.sync.dma_start(dbc[:, :], rep)
                else:
                    dst = dstage_pool.tile([1, D * BL], F16, tag="dst", name="dst")
                    nc.sync.dma_start(dst, d_in["derivt"][step:step + 1, :])
                    gj = pd_pool.tile([1, 1], F32, tag="gj", name="gj")
                    nc.gpsimd.tensor_copy(gj, y32[0:1, 0:1])
                    nc.gpsimd.partition_broadcast(dbc[:, :], dst[:, :])
                    # observer: one tiny op so the DVE sees GPSIMD's sem once
                    # instead of every mult carrying a second wait
                    dj = pd_pool.tile([1, 1], F32, tag="dj", name="dj")
                    nc.vector.tensor_copy(dj, dbc[0:1, 0:1])

                def softplus2(pmm, bias_sb, tag):
                    """softplus over a [128, 2*BL] matmul PSUM tile -> fp16.

                    Zero-bias path keeps ACT to two ops (abs, exp) and folds
                    relu into the final DVE op: g = max(z, 0) + poly(e)."""
                    tb = act_pool.tile([128, 2 * BL], F16, tag=tag + "a", name="spabs")
                    rl = None
                    if zero_b01:
                        nc.scalar.activation(tb, pmm[:, :], AFT.Abs)
                    else:
                        rl = act_pool.tile([128, 2 * BL], F16, tag=tag + "r",
                                           name="sprelu")
                        for x in range(2):
                            sl = slice(x * BL, (x + 1) * BL)
                            nc.scalar.activation(tb[:, sl], pmm[:, sl], AFT.Abs,
                                                 bias=bias_sb[:, x:x + 1])
                            nc.scalar.activation(rl[:, sl], pmm[:, sl], AFT.Relu,
                                                 bias=bias_sb[:, x:x + 1])
                    ee = act_pool.tile([128, 2 * BL], F16, tag=tag + "e", name="spexp")
                    nc.scalar.activation(ee, tb, AFT.Exp, scale=-1.0)
                    pl = act_pool.tile([128, 2 * BL], F16, tag=tag + "p", name="sppoly")
                    nc.vector.tensor_scalar_mul(pl, ee, SP_C[0])
                    for ci in SP_C[1:]:
                        nc.vector.scalar_tensor_tensor(
                            pl, pl, float(ci), ee,
                            op0=mybir.AluOpType.add, op1=mybir.AluOpType.mult)
                    g = act_pool.tile([128, 2 * BL], MM_DTYPE, tag=tag + "g", name="spout")
                    if zero_b01:
                        # g = relu(z) + poly: one DVE op straight off PSUM
                        nc.vector.scalar_tensor_tensor(
                            g, pmm[:, :], 0.0, pl,
                            op0=mybir.AluOpType.max, op1=mybir.AluOpType.add)
                    else:
                        nc.vector.tensor_add(g, pl, rl)
                    return g

                # g1 = softplus(W0 @ y + b0)  (transposed activations)
                pmm = psA.tile([128, 2 * BL], F32, tag="ps_small", name="p_mm0")
                for x in range(2):
                    nc.tensor.matmul(pmm[:, x * BL:(x + 1) * BL],
                                     w0sb[:, x * 128:(x + 1) * 128], ybf[:, :],
                                     start=(x == 0), stop=(x == 1),
                                     skip_group_check=True)
                g1t = softplus2(pmm, b0sb, "sp1")
                g1 = [g1t[:, 0:BL], g1t[:, BL:2 * BL]]
                # g2 = softplus(W1 @ g1 + b1)
                pmm = psA.tile([128, 2 * BL], F32, tag="ps_small", name="p_mm1")
                for x in range(2):
                    for k in range(2):
                        nc.tensor.matmul(
                            pmm[:, x * BL:(x + 1) * BL],
                            w1sb[:, (k * 2 + x) * 128:(k * 2 + x + 1) * 128],
                            g1[k], start=(x == 0 and k == 0),
                            stop=(x == 1 and k == 1), skip_group_check=True)
                g2t = softplus2(pmm, b1sb, "sp2")
                g2 = [g2t[:, 0:BL], g2t[:, BL:2 * BL]]

                # f^T chunks + tanh + einsum, pipelined per PSUM bank.
                # d-major layout throughout: tanh and the deriv multiply are
                # contiguous fp16 (packed 16-bit DVE); only the segmented
                # d-reduce reads strided. One SBUF tile per bank PAIR: sharing
                # a single tile made the pair-01 fold/reduce chain serialize
                # behind banks 2/3's tanh writes (tile-granular cross-engine
                # ordering), pushing ~0.8us of "hidden" work onto the tail.
                ft01 = ft_pool.tile([128, 2 * JPB * BL], F16, tag="ft01", name="ft01")
                ft23 = ft_pool.tile([128, 2 * JPB * BL], F16, tag="ft23", name="ft23")
                for bank in range(N_BANKS):
                    pf = psF.tile([128, JPB * BL], F32, tag="ftbank", name="pf")
                    # start=True only on the very first MM: it arms the whole
                    # 2KB bank's has_written clear, so later chunks overwrite
                    # their own columns and the final bias MM accumulates.
                    for j16 in range(JPB):
                        j = bank * JPB + j16
                        o = j16 * BL
                        for k in range(2):
                            last = skip_b2 and j16 == JPB - 1 and k == 1
                            nc.tensor.matmul(
                                pf[:, o:o + BL],
                                w2sb[:, (k * NJ + j) * 128:(k * NJ + j + 1) * 128],
                                g2[k], start=(j16 == 0 and k == 0), stop=last,
                                skip_group_check=True)
                    if not skip_b2:
                        # + b2 via one-hot matmul accumulation (keeps the bias
                        # add on the PE so tanh only ever waits on the PE sem)
                        nc.tensor.matmul(pf[:, :],
                                         b2sb[:, bank * 128:(bank + 1) * 128],
                                         onehot[:, :], start=False, stop=True,
                                         skip_group_check=True)
                    ft = ft01 if bank < 2 else ft23
                    fo = (bank % 2) * JPB * BL
                    bo = bank * JPB * BL
                    fsl = ft[:, fo:fo + JPB * BL]
                    nc.scalar.activation(fsl, pf[:, :], AFT.Tanh)
                    if bank < 2:
                        # * deriv (in place, contiguous packed fp16); banks
                        # 2/3 multiply in one wide op on the tail instead --
                        # a per-bank mult2 competed with the pair-01 fold
                        # chain for the 1.7us DVE overlap window.
                        nc.vector.tensor_mul(fsl, fsl, dbc[:, bo:bo + JPB * BL])
                    if bank == 1:
                        # bank-pair 01: fold d-halves contiguously (sum over
                        # d is order-free), segmented-reduce the remaining 8
                        # d's, and fold into y -- all hidden under the PE's
                        # bank-2/3 matmuls and tanh windows.
                        q = JPB * BL  # 512
                        nc.vector.tensor_add(ft01[:, 0:q], ft01[:, 0:q],
                                             ft01[:, q:2 * q])
                        nc.vector.tensor_add(ft01[:, 0:q // 2], ft01[:, 0:q // 2],
                                             ft01[:, q // 2:q])
                        pd01 = pd_pool.tile([128, BL], F32, tag="pd01", name="pd01")
                        nc.vector.tensor_reduce(
                            pd01, ft01[:, 0:q // 2].rearrange(
                                "p (d b) -> p b d", b=BL),
                            axis=mybir.AxisListType.X, op=mybir.AluOpType.add)
                        nc.vector.tensor_add(y32, y32, pd01)
                # bank-pair 23: one wide deriv multiply, then fold + reduce
                # on the step's critical tail
                q = JPB * BL
                nc.vector.tensor_mul(ft23[:, 0:2 * q], ft23[:, 0:2 * q],
                                     dbc[:, 2 * q:4 * q])
                nc.vector.tensor_add(ft23[:, 0:q], ft23[:, 0:q],
                                     ft23[:, q:2 * q])
                nc.vector.tensor_add(ft23[:, 0:q // 2], ft23[:, 0:q // 2],
                                     ft23[:, q // 2:q])
                pd23 = pd_pool.tile([128, BL], F32, tag="pd23", name="pd23")
                nc.vector.tensor_reduce(
                    pd23, ft23[:, 0:q // 2].rearrange(
                        "p (d b) -> p b d", b=BL),
                    axis=mybir.AxisListType.X, op=mybir.AluOpType.add)
                # emit the fp16 copy FIRST so mm0 of the next step can start
                # one DVE op earlier; the f32 accumulator update follows
                # off the critical path.
                nc.vector.tensor_add(ybf, y32, pd23)
                nc.vector.tensor_add(y32, y32, pd23)

            # --- readout: sigmoid(z) = 0.5 + 0.5*tanh(0.5*z + 0.5*l_b) ---
            po = psW.tile([1, BL], F32, tag="wjunk", name="p_out")
            nc.tensor.matmul(po, lwsb[:, :], y32[:, :],
                             start=True, stop=True)
            tnh = cpool.tile([1, BL], F32, name="tnh")
            nc.scalar.activation(tnh, po, AFT.Tanh, bias=lbsb[:, 0:1], scale=0.5)
            osb = cpool.tile([1, BL], F32, name="osb")
            nc.vector.tensor_scalar(osb, tnh, 0.5, 0.5,
                                    op0=mybir.AluOpType.mult,
                                    op1=mybir.AluOpType.add)
            nc.sync.dma_start(out_dram, osb)

    return nc


class Runner:
    """Compile once; execute repeatedly with device-resident inputs.

    Mirrors bass2jax.run_bass_via_pjrt's multi-core shard_map path but keeps
    the jitted executable and the H2D-transferred inputs so warm invocations
    measure (dispatch + NEFF execution) only.
    """

    def __init__(self, nc, in_maps):
        import jax
        from jax.sharding import Mesh, PartitionSpec
        from jax.experimental.shard_map import shard_map
        from concourse import bass2jax, mybir as mb

        bass2jax.install_neuronx_cc_hook()
        n_cores = len(in_maps)
        assert nc.partition_id_tensor is None and nc.dbg_addr is None

        in_names, out_names, out_avals, zero_outs = [], [], [], []
        for alloc in nc.m.functions[0].allocations:
            if not isinstance(alloc, mb.MemoryLocationSet):
                continue
            name = alloc.memorylocations[0].name
            if alloc.kind == "ExternalInput":
                in_names.append(name)
            elif alloc.kind == "ExternalOutput":
                shape = tuple(alloc.tensor_shape)
                dtype = mb.dt.np(alloc.dtype)
                out_names.append(name)
                out_avals.append(jax.core.ShapedArray(shape, dtype))
                zero_outs.append(np.zeros(shape, dtype))
        n_params = len(in_names)
        all_in_names = tuple(in_names + out_names)

        def _body(*args):
            outs = bass2jax._bass_exec_p.bind(
                *args,
                out_avals=tuple(out_avals),
                in_names=all_in_names,
                out_names=tuple(out_names),
                lowering_input_output_aliases=(),
                sim_require_finite=True,
                sim_require_nnan=True,
                nc=nc,
            )
            return tuple(outs)

        devices = jax.devices()[:n_cores]
        mesh = Mesh(np.asarray(devices), ("core",))
        n_outs = len(out_names)

        self._sharded = jax.jit(
            shard_map(_body, mesh=mesh,
                      in_specs=(PartitionSpec("core"),) * (n_params + n_outs),
                      out_specs=(PartitionSpec("core"),) * n_outs,
                      check_rep=False),
            donate_argnums=tuple(range(n_params, n_params + n_outs)),
            keep_unused=True)
        concat_in = [
            np.concatenate([np.asarray(in_maps[c][nm]) for c in range(n_cores)], axis=0)
            for nm in in_names]
        self._dev_in = [jax.device_put(
            a, jax.sharding.NamedSharding(mesh, PartitionSpec("core")))
            for a in concat_in]
        self._zero_shapes = [(n_cores * z.shape[0], *z.shape[1:]) for z in zero_outs]
        self._zero_dtypes = [z.dtype for z in zero_outs]
        self._out_names = out_names
        self._out_avals = out_avals
        self._n_cores = n_cores
        self._jax = jax

    def __call__(self):
        zeros = [np.zeros(s, d) for s, d in zip(self._zero_shapes, self._zero_dtypes)]
        outs = self._sharded(*self._dev_in, *zeros)
        # np.asarray blocks on the device result itself; an explicit
        # block_until_ready first would cost a second tunnel round trip.
        outs = [np.asarray(o) for o in outs]
        return [
            {nm: outs[i].reshape(self._n_cores, *self._out_avals[i].shape)[c]
             for i, nm in enumerate(self._out_names)}
            for c in range(self._n_cores)
        ]


def make_runner(inputs, n_steps=None):
    per_core = _prep_host(inputs)
    skip_b2 = bool(np.all(np.asarray(inputs["f_b2"]) == 0.0))
    zero_b01 = bool(np.all(np.asarray(inputs["f_b0"]) == 0.0)
                    and np.all(np.asarray(inputs["f_b1"]) == 0.0))
    nc = build_program(N_STEPS if n_steps is None else n_steps, skip_b2=skip_b2,
                       zero_b01=zero_b01)
    # codegen-level only (CoreSim can't ingest post-hoc instructions)
    _legalize_waits(nc)
    return Runner(nc, per_core)


def run(inputs):
    """Build + run on the 8 NeuronCores; returns output [256]."""
    runner = make_runner(inputs)
    results = runner()
    outs = [results[i]["out"].reshape(BL) for i in range(N_CORES)]
    return np.concatenate(outs).astype(np.float32)


def kernel(**inputs):
    return run(inputs)





# revision 3
# speedup vs baseline: 8.0751x; 8.0751x over previous
"""Trainium2 Bass kernel for a Neural CDE (Euler scan over cubic-spline control).

Strategy
--------
Data-parallel over batch: B=256 -> 8 cores x 32. The T=512 Euler steps are
inherently sequential (y_{k+1} depends on y_k), so each core runs the full
scan on its batch shard with all MLP weights resident in SBUF.

Per step the dominant op is f = tanh(g2 @ f_w2.T + b2) with f_w2 [8192, 256].
We run it weights-stationary on the PE (fp16 weights -> fast-weight-load),
emitting f TRANSPOSED as 64 chunks [128(h), 32(b)], one chunk per spline
dim d (f_w2 rows are pre-permuted d-major on the host). That layout gives
full 128-partition utilization for the tanh (ACT) and lets the einsum
  y += sum_d f[:,h,d] * deriv[:,d]
become one contiguous packed-fp16 DVE multiply by a partition-broadcast
deriv row plus a strided segmented reduce.

The cubic-spline derivative deriv_k is state-independent, so it is
precomputed on the host (exactly mirroring the reference fp32 arithmetic),
pre-transposed d-major in fp16, and replicated across partitions by the
(otherwise idle) DMA engines via a stride-0 source read, ahead of use.
"""

import os
import sys

import numpy as np

for _p in ("/opt/trn_rl_repo",):
    if _p not in sys.path:
        sys.path.insert(0, _p)

import ml_dtypes  # noqa: E402

import concourse.bass as bass  # noqa: E402
import concourse.tile as tile  # noqa: E402
from concourse import library_config, mybir  # noqa: E402
from concourse.bass_utils import run_bass_kernel_spmd  # noqa: E402

B, T, D, H, W = 256, 512, 64, 128, 256
N_CORES = 8
BL = B // N_CORES  # 32 batch per core
HD = H * D  # 8192
NJ = D  # 64 output chunks of [128, 32], one per spline dim d
N_BANKS = 4  # PSUM banks holding f^T: 16 chunks x [128,32] each
JPB = NJ // N_BANKS  # 16 chunks per bank

F32 = mybir.dt.float32
F32R = mybir.dt.float32r
BF16 = mybir.dt.bfloat16
F16 = mybir.dt.float16
AFT = mybir.ActivationFunctionType

# Matmul dtype for the three per-step MLP matmuls (weights & activations).
# fp16: same PE/DVE/FWL speed as bf16 but 10 mantissa bits -- the weights
# (<=1/16), activations, tanh outputs and spline derivs are all well inside
# fp16 range, so quantization noise drops ~8x vs bf16.
MM_DTYPE = F16
# Steps actually emitted (patchable for fast simulation tests).
N_STEPS = int(os.environ.get("K_STEPS", T))
# deriv broadcast mechanism: dma (stride-0 partition-broadcast DMA) | gpsimd
BCAST = os.environ.get("K_BCAST", "dma")

# softplus(x) = relu(x) + log1p(exp(-|x|)); the walrus ACT tables here have
# no softplus set that coexists with tanh, so log1p is a minimax polynomial
# sum_{i=1..3} c_i e^i on e in [0,1], evaluated on the DVE via the
# (p + c_i) * e recurrence. Coefficients c3..c1 (max err ~1.1e-3 absolute,
# comparable to the fp16 rounding of g itself; each extra degree costs a
# serial ~220ns DVE op on the per-step head chain twice).
SP_C = [0.12855132, -0.42702028, 0.9926641]


def _f32(x):
    return np.ascontiguousarray(x, dtype=np.float32)


def _prep_host(inputs):
    """Host-side layout prep: shard batch, transpose weights into lhsT tile
    layouts, precompute the spline derivative for every step (exact fp32
    mirror of the reference's searchsorted/frac arithmetic)."""
    ts = np.asarray(inputs["ts"], np.float32)
    ca = np.asarray(inputs["coeffs_a"], np.float32)
    cb = np.asarray(inputs["coeffs_b"], np.float32)
    cc = np.asarray(inputs["coeffs_c"], np.float32)
    cd = np.asarray(inputs["coeffs_d"], np.float32)

    dt = np.float32(ts[1] - ts[0])
    # t carry exactly as the reference accumulates it (fp32 adds)
    t_seq = np.empty(T, np.float32)
    t_seq[0] = ts[0]
    for k in range(1, T):
        t_seq[k] = np.float32(t_seq[k - 1] + dt)
    idx = np.clip(np.searchsorted(ts, t_seq, side="right") - 1, 0, T - 2)
    frac = (t_seq - ts[idx]).astype(np.float32)
    # deriv[k] = b[:,idx] + 2*c[:,idx]*frac + 3*d[:,idx]*frac^2   [B, T, D]
    f1 = frac[None, :, None]
    deriv = cb[:, idx, :] + np.float32(2.0) * cc[:, idx, :] * f1 \
        + np.float32(3.0) * cd[:, idx, :] * (f1 * f1)
    deriv = _f32(deriv * dt)  # fold the Euler dt (dt==1.0 -> exact)

    w0 = np.asarray(inputs["f_w0"], np.float32)  # [W=256, H=128]
    w1 = np.asarray(inputs["f_w1"], np.float32)  # [256, 256]
    w2 = np.asarray(inputs["f_w2"], np.float32)  # [8192, 256]
    b0 = np.asarray(inputs["f_b0"], np.float32)
    b1 = np.asarray(inputs["f_b1"], np.float32)
    b2 = np.asarray(inputs["f_b2"], np.float32)
    iw0 = np.asarray(inputs["i_w0"], np.float32)  # [256, 64]
    iw1 = np.asarray(inputs["i_w1"], np.float32)  # [256, 256]
    iw2 = np.asarray(inputs["i_w2"], np.float32)  # [128, 256]
    ib0 = np.asarray(inputs["i_b0"], np.float32)
    ib1 = np.asarray(inputs["i_b1"], np.float32)
    ib2 = np.asarray(inputs["i_b2"], np.float32)
    lw = np.asarray(inputs["l_w"], np.float32)  # [1, 128]
    lb = np.asarray(inputs["l_b"], np.float32)  # [1]

    mmnp = (np.float16 if MM_DTYPE == F16 else
            ml_dtypes.bfloat16 if MM_DTYPE == BF16 else np.float32)

    shared = {
        # func MLP, lhsT tile layouts ([K, M] per 128-col group)
        "w0": np.ascontiguousarray(w0.T, dtype=mmnp),  # [128, 256]
        "w1": np.ascontiguousarray(
            w1.reshape(2, 128, 2, 128).transpose(3, 2, 0, 1).reshape(128, 512),
            dtype=mmnp),
        # d-major chunk permutation: tile (kappa, j) cols -> f_w2[m*64+j, kappa*128+k]
        "w2": np.ascontiguousarray(
            w2.reshape(128, 64, 2, 128).transpose(3, 2, 1, 0).reshape(128, 16384),
            dtype=mmnp),
        "b0": _f32(b0.reshape(2, 128).T),  # [128, 2]
        "b1": _f32(b1.reshape(2, 128).T),
        # b2 added in PSUM by the PE via a one-hot matmul: for bank q,
        # psum[m, j16*32+b] += sum_j' b2mm[j', q*128+m] * onehot[j', j16*32+b]
        "b2mm": np.ascontiguousarray(
            b2.reshape(128, 4, 16).transpose(2, 1, 0).reshape(16, 512), dtype=mmnp),
        "onehot": np.ascontiguousarray(
            np.repeat(np.eye(16, dtype=np.float32), 32, axis=1), dtype=mmnp),
        # initial MLP (run once, f32r)
        "iw0": _f32(iw0.T),  # [64, 256]
        "iw1": _f32(iw1.reshape(2, 128, 2, 128).transpose(3, 2, 0, 1).reshape(128, 512)),
        "iw2": _f32(iw2.reshape(128, 2, 128).transpose(2, 1, 0).reshape(128, 256)),
        "ib0": _f32(ib0.reshape(2, 128).T),
        "ib1": _f32(ib1.reshape(2, 128).T),
        "ib2": _f32(ib2.reshape(128, 1)),
        "lw": _f32(lw.reshape(128, 1)),
        # pre-halved: sigmoid(z) = 0.5 + 0.5*tanh(0.5*z + 0.5*l_b)
        "lb": _f32(lb.reshape(1, 1) * 0.5),
    }

    per_core = []
    for s in range(N_CORES):
        sl = slice(s * BL, (s + 1) * BL)
        m = dict(shared)
        # x0 = coeffs_a[:, 0, :] transposed -> [64, 32]
        m["x0t"] = _f32(ca[sl, 0, :].T)
        # deriv per step, d-major (matching the f^T chunk layout) in fp16:
        # one [1, 2048] row per step; broadcast to 128 partitions off the DVE.
        m["derivt"] = np.ascontiguousarray(
            deriv[sl].transpose(1, 2, 0).reshape(T, D * BL), dtype=np.float16)
        per_core.append(m)
    return per_core


def _legalize_waits(nc, max_waits=1):
    """This walrus build allows only one embedded sem-wait on most
    instruction encodings. Spill extra waits onto standalone same-engine
    EventSemaphore instructions placed immediately before."""
    n_spilled = 0
    for f in nc.m.functions:
        for blk in f.blocks:
            out = []
            for inst in blk.instructions:
                si = inst.sync_info
                if si is not None and len(si.on_wait) > max_waits:
                    waits = list(si.on_wait)
                    for j, w in enumerate(waits[:-max_waits]):
                        ev = mybir.InstEventSemaphore(
                            name=f"{inst.name}-w{j}", engine=inst.engine,
                            ins=[], outs=[],
                            sync_info=mybir.SyncInfo(on_wait=[w], on_update=[]))
                        out.append(ev)
                        n_spilled += 1
                    inst.sync_info = mybir.SyncInfo(
                        on_wait=waits[-max_waits:], on_update=list(si.on_update))
                out.append(inst)
            blk.instructions = out
    return n_spilled


def build_program(n_steps=None, skip_b2=False, zero_b01=False):
    """Build the single-core Bass/Tile program (same program on all cores).

    skip_b2: omit the per-bank one-hot bias matmuls (valid when f_b2 == 0,
    which make_runner detects from the actual input data).
    zero_b01: f_b0 == f_b1 == 0 -> merged [128, 64] softplus head ops
    (ACT bias is per-partition, so halves with distinct biases can't merge)."""
    if n_steps is None:
        n_steps = N_STEPS
    nc = bass.Bass("TRN2", target_bir_lowering=False, debug=False,
                   enable_asserts=False, num_devices=N_CORES,
                   enable_partition_id=False)

    d_in = {}
    for name, shape, dtyp in [
        ("x0t", [D, BL], F32),
        ("derivt", [T, D * BL], F16),
        ("w0", [128, 256], MM_DTYPE),
        ("w1", [128, 512], MM_DTYPE),
        ("w2", [128, 16384], MM_DTYPE),
        ("b0", [128, 2], F32),
        ("b1", [128, 2], F32),
        ("b2mm", [16, 512], MM_DTYPE),
        ("onehot", [16, 512], MM_DTYPE),
        ("iw0", [64, 256], F32),
        ("iw1", [128, 512], F32),
        ("iw2", [128, 256], F32),
        ("ib0", [128, 2], F32),
        ("ib1", [128, 2], F32),
        ("ib2", [128, 1], F32),
        ("lw", [128, 1], F32),
        ("lb", [1, 1], F32),
    ]:
        d_in[name] = nc.dram_tensor(name, shape, dtyp, kind="ExternalInput").ap()
    out_dram = nc.dram_tensor("out", [1, BL], F32, kind="ExternalOutput").ap()

    with tile.TileContext(nc) as tc:
        from contextlib import ExitStack
        ctx = ExitStack()
        with ctx:
            cpool = ctx.enter_context(tc.tile_pool(name="const", bufs=1))
            psW = ctx.enter_context(tc.tile_pool(name="psW", bufs=1, space="PSUM"))
            psA = ctx.enter_context(tc.tile_pool(name="psA", bufs=3, space="PSUM"))
            psF = ctx.enter_context(tc.tile_pool(name="psF", bufs=4, space="PSUM"))
            dstage_pool = ctx.enter_context(tc.tile_pool(name="dstage", bufs=3))
            dbc_pool = ctx.enter_context(tc.tile_pool(name="dbc", bufs=4))
            ft_pool = ctx.enter_context(tc.tile_pool(name="ftp", bufs=2))
            act_pool = ctx.enter_context(tc.tile_pool(name="actp", bufs=2))
            pd_pool = ctx.enter_context(tc.tile_pool(name="pdp", bufs=8))

            if BCAST == "gpsimd":
                # partition_broadcast lives in the 'mlp' GPSIMD ucode library
                nc.gpsimd.load_library(library_config.mlp)

            # --- persistent SBUF tensors ---
            w0sb = cpool.tile_from(d_in["w0"], name="w0sb")
            w1sb = cpool.tile_from(d_in["w1"], name="w1sb")
            w2sb = cpool.tile_from(d_in["w2"], name="w2sb")
            b0sb = cpool.tile_from(d_in["b0"], name="b0sb")
            b1sb = cpool.tile_from(d_in["b1"], name="b1sb")
            b2sb = cpool.tile_from(d_in["b2mm"], name="b2sb")
            onehot = cpool.tile_from(d_in["onehot"], name="onehot")
            iw0sb = cpool.tile_from(d_in["iw0"], name="iw0sb")
            iw1sb = cpool.tile_from(d_in["iw1"], name="iw1sb")
            iw2sb = cpool.tile_from(d_in["iw2"], name="iw2sb")
            ib0sb = cpool.tile_from(d_in["ib0"], name="ib0sb")
            ib1sb = cpool.tile_from(d_in["ib1"], name="ib1sb")
            ib2sb = cpool.tile_from(d_in["ib2"], name="ib2sb")
            lwsb = cpool.tile_from(d_in["lw"], name="lwsb")
            lbsb = cpool.tile_from(d_in["lb"], name="lbsb")
            x0sb = cpool.tile_from(d_in["x0t"], name="x0sb")

            y32 = cpool.tile([128, BL], F32, name="y32")
            ybf = cpool.tile([128, BL], MM_DTYPE, name="ybf")

            # Warm each compute engine's vector clock over the const DMAs so
            # later instructions don't accumulate multiple sem waits (the
            # bias-carrying ACT encoding has room for only one).
            for wi, t in enumerate((b0sb, b1sb, ib0sb, ib1sb, ib2sb)):
                w = cpool.tile([128, 1], F32, name=f"warma{wi}")
                nc.scalar.copy(w, t[:, 0:1])
            warml = cpool.tile([1, 1], F32, name="warml")
            nc.scalar.copy(warml, lbsb[:, 0:1])
            for wi, t in enumerate((b0sb, b1sb, x0sb)):
                w = cpool.tile([t.shape[0], 1], F32, name=f"warmv{wi}")
                nc.vector.tensor_copy(w, t[:, 0:1])
            # PE: one junk self-pair matmul per weight DMA so real matmuls
            # never wait on more than one fresh semaphore.
            wjunk = psW.tile([128, 2], F32, name="wjunk")
            warm_mms = [x0sb, iw0sb, iw1sb, iw2sb, lwsb, w0sb, w1sb, b2sb, onehot]
            warm_mms += [w2sb[:, c:c + 1024] for c in range(0, 16384, 1024)]
            for t in warm_mms:
                mm = min(t.free_size(), 64)
                nc.tensor.matmul(wjunk[:mm, 0:1], t[:, 0:mm], t[:, 0:1],
                                 start=True, stop=True)

            # --- initial MLP: y0 = I2 @ relu(I1 @ relu(I0 @ x0)) (f32r) ---
            h0 = [None, None]
            for x in range(2):
                p = psA.tile([128, BL], F32, tag="ps_small", name="p_init0")
                nc.tensor.matmul(p, iw0sb[:, x * 128:(x + 1) * 128],
                                 x0sb[:, :], start=True, stop=True)
                h = act_pool.tile([128, BL], F32, tag="h_init", name="h_init0")
                nc.scalar.activation(h, p, AFT.Relu, bias=ib0sb[:, x:x + 1])
                h0[x] = h
            h1 = [None, None]
            for x in range(2):
                p = psA.tile([128, BL], F32, tag="ps_small", name="p_init1")
                for k in range(2):
                    nc.tensor.matmul(p, iw1sb[:, (k * 2 + x) * 128:(k * 2 + x + 1) * 128],
                                     h0[k][:, :], start=(k == 0), stop=(k == 1))
                h = act_pool.tile([128, BL], F32, tag="h_init2", name="h_init1")
                nc.scalar.activation(h, p, AFT.Relu, bias=ib1sb[:, x:x + 1])
                h1[x] = h
            p = psA.tile([128, BL], F32, tag="ps_small", name="p_init2")
            for k in range(2):
                nc.tensor.matmul(p, iw2sb[:, k * 128:(k + 1) * 128],
                                 h1[k][:, :], start=(k == 0), stop=(k == 1))
            nc.scalar.activation(y32, p, AFT.Identity, bias=ib2sb[:, 0:1])
            nc.scalar.copy(ybf, y32)

            # --- the scan ---
            for step in range(n_steps):
                # deriv row for this step: replicate the fp16 [1, 2048] DRAM
                # row to all 128 partitions in one DMA via a stride-0 repeat
                # on the source free axis. Reading the row 128x from HBM
                # parallelizes across banks/queues (a single-partition SBUF
                # source would serialize on one read port).
                dbc = dbc_pool.tile([128, D * BL], F16, tag="dbc", name="dbc")
                if BCAST == "dma":
                    row = d_in["derivt"][step:step + 1, :]
                    rep = bass.AP(row.tensor, row.offset,
                                  [list(row.ap[0]), [0, 128], [1, D * BL]])
                    nc.sync.dma_start(dbc[:, :], rep)
                else:
                    dst = dstage_pool.tile([1, D * BL], F16, tag="dst", name="dst")
                    nc.sync.dma_start(dst, d_in["derivt"][step:step + 1, :])
                    gj = pd_pool.tile([1, 1], F32, tag="gj", name="gj")
                    nc.gpsimd.tensor_copy(gj, y32[0:1, 0:1])
                    nc.gpsimd.partition_broadcast(dbc[:, :], dst[:, :])
                    # observer: one tiny op so the DVE sees GPSIMD's sem once
                    # instead of every mult carrying a second wait
                    dj = pd_pool.tile([1, 1], F32, tag="dj", name="dj")
                    nc.vector.tensor_copy(dj, dbc[0:1, 0:1])

                def softplus2(pmm, bias_sb, tag):
                    """softplus over a [128, 2*BL] matmul PSUM tile -> fp16.

                    Zero-bias path keeps ACT to two ops (abs, exp) and folds
                    relu into the final DVE op: g = max(z, 0) + poly(e)."""
                    tb = act_pool.tile([128, 2 * BL], F16, tag=tag + "a", name="spabs")
                    rl = None
                    if zero_b01:
                        nc.scalar.activation(tb, pmm[:, :], AFT.Abs)
                    else:
                        rl = act_pool.tile([128, 2 * BL], F16, tag=tag + "r",
                                           name="sprelu")
                        for x in range(2):
                            sl = slice(x * BL, (x + 1) * BL)
                            nc.scalar.activation(tb[:, sl], pmm[:, sl], AFT.Abs,
                                                 bias=bias_sb[:, x:x + 1])
                            nc.scalar.activation(rl[:, sl], pmm[:, sl], AFT.Relu,
                                                 bias=bias_sb[:, x:x + 1])
                    ee = act_pool.tile([128, 2 * BL], F16, tag=tag + "e", name="spexp")
                    nc.scalar.activation(ee, tb, AFT.Exp, scale=-1.0)
                    pl = act_pool.tile([128, 2 * BL], F16, tag=tag + "p", name="sppoly")
                    nc.vector.tensor_scalar_mul(pl, ee, SP_C[0])
                    for ci in SP_C[1:]:
                        nc.vector.scalar_tensor_tensor(
                            pl, pl, float(ci), ee,
                            op0=mybir.AluOpType.add, op1=mybir.AluOpType.mult)
                    g = act_pool.tile([128, 2 * BL], MM_DTYPE, tag=tag + "g", name="spout")
                    if zero_b01:
                        # g = relu(z) + poly: one DVE op straight off PSUM
                        nc.vector.scalar_tensor_tensor(
                            g, pmm[:, :], 0.0, pl,
                            op0=mybir.AluOpType.max, op1=mybir.AluOpType.add)
                    else:
                        nc.vector.tensor_add(g, pl, rl)
                    return g

                # g1 = softplus(W0 @ y + b0)  (transposed activations)
                pmm = psA.tile([128, 2 * BL], F32, tag="ps_small", name="p_mm0")
                for x in range(2):
                    nc.tensor.matmul(pmm[:, x * BL:(x + 1) * BL],
                                     w0sb[:, x * 128:(x + 1) * 128], ybf[:, :],
                                     start=(x == 0), stop=(x == 1),
                                     skip_group_check=True)
                g1t = softplus2(pmm, b0sb, "sp1")
                g1 = [g1t[:, 0:BL], g1t[:, BL:2 * BL]]
                # g2 = softplus(W1 @ g1 + b1)
                pmm = psA.tile([128, 2 * BL], F32, tag="ps_small", name="p_mm1")
                for x in range(2):
                    for k in range(2):
                        nc.tensor.matmul(
                            pmm[:, x * BL:(x + 1) * BL],
                            w1sb[:, (k * 2 + x) * 128:(k * 2 + x + 1) * 128],
                            g1[k], start=(x == 0 and k == 0),
                            stop=(x == 1 and k == 1), skip_group_check=True)
                g2t = softplus2(pmm, b1sb, "sp2")
                g2 = [g2t[:, 0:BL], g2t[:, BL:2 * BL]]

                # f^T chunks + tanh + einsum, pipelined per PSUM bank.
                # d-major layout throughout: tanh and the deriv multiply are
                # contiguous fp16 (packed 16-bit DVE); only the segmented
                # d-reduce reads strided. One SBUF tile per bank PAIR: sharing
                # a single tile made the pair-01 fold/reduce chain serialize
                # behind banks 2/3's tanh writes (tile-granular cross-engine
                # ordering), pushing ~0.8us of "hidden" work onto the tail.
                ft01 = ft_pool.tile([128, 2 * JPB * BL], F16, tag="ft01", name="ft01")
                ft23 = ft_pool.tile([128, 2 * JPB * BL], F16, tag="ft23", name="ft23")
                for bank in range(N_BANKS):
                    pf = psF.tile([128, JPB * BL], F32, tag="ftbank", name="pf")
                    # start=True only on the very first MM: it arms the whole
                    # 2KB bank's has_written clear, so later chunks overwrite
                    # their own columns and the final bias MM accumulates.
                    for j16 in range(JPB):
                        j = bank * JPB + j16
                        o = j16 * BL
                        for k in range(2):
                            last = skip_b2 and j16 == JPB - 1 and k == 1
                            nc.tensor.matmul(
                                pf[:, o:o + BL],
                                w2sb[:, (k * NJ + j) * 128:(k * NJ + j + 1) * 128],
                                g2[k], start=(j16 == 0 and k == 0), stop=last,
                                skip_group_check=True)
                    if not skip_b2:
                        # + b2 via one-hot matmul accumulation (keeps the bias
                        # add on the PE so tanh only ever waits on the PE sem)
                        nc.tensor.matmul(pf[:, :],
                                         b2sb[:, bank * 128:(bank + 1) * 128],
                                         onehot[:, :], start=False, stop=True,
                                         skip_group_check=True)
                    ft = ft01 if bank < 2 else ft23
                    fo = (bank % 2) * JPB * BL
                    bo = bank * JPB * BL
                    fsl = ft[:, fo:fo + JPB * BL]
                    nc.scalar.activation(fsl, pf[:, :], AFT.Tanh)
                    if bank < 2:
                        # * deriv (in place, contiguous packed fp16); banks
                        # 2/3 multiply in one wide op on the tail instead --
                        # a per-bank mult2 competed with the pair-01 fold
                        # chain for the 1.7us DVE overlap window.
                        nc.vector.tensor_mul(fsl, fsl, dbc[:, bo:bo + JPB * BL])
                    if bank == 1:
                        # bank-pair 01: fold d-halves contiguously (sum over
                        # d is order-free), segmented-reduce the remaining 8
                        # d's, and fold into y -- all hidden under the PE's
                        # bank-2/3 matmuls and tanh windows.
                        q = JPB * BL  # 512
                        nc.vector.tensor_add(ft01[:, 0:q], ft01[:, 0:q],
                                             ft01[:, q:2 * q])
                        nc.vector.tensor_add(ft01[:, 0:q // 2], ft01[:, 0:q // 2],
                                             ft01[:, q // 2:q])
                        pd01 = pd_pool.tile([128, BL], F32, tag="pd01", name="pd01")
                        nc.vector.tensor_reduce(
                            pd01, ft01[:, 0:q // 2].rearrange(
                                "p (d b) -> p b d", b=BL),
                            axis=mybir.AxisListType.X, op=mybir.AluOpType.add)
                        nc.vector.tensor_add(y32, y32, pd01)
                # bank-pair 23: one wide deriv multiply, then fold + reduce
                # on the step's critical tail
                q = JPB * BL
                nc.vector.tensor_mul(ft23[:, 0:2 * q], ft23[:, 0:2 * q],
                                     dbc[:, 2 * q:4 * q])
                nc.vector.tensor_add(ft23[:, 0:q], ft23[:, 0:q],
                                     ft23[:, q:2 * q])
                nc.vector.tensor_add(ft23[:, 0:q // 2], ft23[:, 0:q // 2],
                                     ft23[:, q // 2:q])
                pd23 = pd_pool.tile([128, BL], F32, tag="pd23", name="pd23")
                nc.vector.tensor_reduce(
                    pd23, ft23[:, 0:q // 2].rearrange(
                        "p (d b) -> p b d", b=BL),
                    axis=mybir.AxisListType.X, op=mybir.AluOpType.add)
                # emit the fp16 copy FIRST so mm0 of the next step can start
                # one DVE op earlier; the f32 accumulator update follows
                # off the critical path.
                nc.vector.tensor_add(ybf, y32, pd23)
                nc.vector.tensor_add(y32, y32, pd23)

            # --- readout: sigmoid(z) = 0.5 + 0.5*tanh(0.5*z + 0.5*l_b) ---
            po = psW.tile([1, BL], F32, tag="wjunk", name="p_out")
            nc.tensor.matmul(po, lwsb[:, :], y32[:, :],
                             start=True, stop=True)
            tnh = cpool.tile([1, BL], F32, name="tnh")
            nc.scalar.activation(tnh, po, AFT.Tanh, bias=lbsb[:, 0:1], scale=0.5)
            osb = cpool.tile([1, BL], F32, name="osb")
            nc.vector.tensor_scalar(osb, tnh, 0.5, 0.5,
                                    op0=mybir.AluOpType.mult,
                                    op1=mybir.AluOpType.add)
            nc.sync.dma_start(out_dram, osb)

    return nc


class Runner:
    """Compile once; execute repeatedly with device-resident inputs.

    Mirrors bass2jax.run_bass_via_pjrt's multi-core shard_map path but keeps
    the jitted executable and the H2D-transferred inputs so warm invocations
    measure (dispatch + NEFF execution) only.
    """

    def __init__(self, nc, in_maps):
        import jax
        from jax.sharding import Mesh, PartitionSpec
        from jax.experimental.shard_map import shard_map
        from concourse import bass2jax, mybir as mb

        bass2jax.install_neuronx_cc_hook()
        n_cores = len(in_maps)
        assert nc.partition_id_tensor is None and nc.dbg_addr is None

        in_names, out_names, out_avals, zero_outs = [], [], [], []
        for alloc in nc.m.functions[0].allocations:
            if not isinstance(alloc, mb.MemoryLocationSet):
                continue
            name = alloc.memorylocations[0].name
            if alloc.kind == "ExternalInput":
                in_names.append(name)
            elif alloc.kind == "ExternalOutput":
                shape = tuple(alloc.tensor_shape)
                dtype = mb.dt.np(alloc.dtype)
                out_names.append(name)
                out_avals.append(jax.core.ShapedArray(shape, dtype))
                zero_outs.append(np.zeros(shape, dtype))
        n_params = len(in_names)
        all_in_names = tuple(in_names + out_names)

        def _body(*args):
            outs = bass2jax._bass_exec_p.bind(
                *args,
                out_avals=tuple(out_avals),
                in_names=all_in_names,
                out_names=tuple(out_names),
                lowering_input_output_aliases=(),
                sim_require_finite=True,
                sim_require_nnan=True,
                nc=nc,
            )
            return tuple(outs)

        devices = jax.devices()[:n_cores]
        mesh = Mesh(np.asarray(devices), ("core",))
        n_outs = len(out_names)

        # No donation: the kernel writes the full output tensor, so the
        # zero out-buffers can stay device-resident and be reused every
        # call.  (Donated zeros were re-uploaded through the axon tunnel
        # on every invocation, ~3 ms of host bookkeeping per call.)
        self._sharded = jax.jit(
            shard_map(_body, mesh=mesh,
                      in_specs=(PartitionSpec("core"),) * (n_params + n_outs),
                      out_specs=(PartitionSpec("core"),) * n_outs,
                      check_rep=False),
            keep_unused=True)
        sh = jax.sharding.NamedSharding(mesh, PartitionSpec("core"))
        concat_in = [
            np.concatenate([np.asarray(in_maps[c][nm]) for c in range(n_cores)], axis=0)
            for nm in in_names]
        self._dev_in = [jax.device_put(a, sh) for a in concat_in]
        self._zero_shapes = [(n_cores * z.shape[0], *z.shape[1:]) for z in zero_outs]
        self._zero_dtypes = [z.dtype for z in zero_outs]
        self._dev_zeros = [
            jax.device_put(np.zeros(s, d), sh)
            for s, d in zip(self._zero_shapes, self._zero_dtypes)]
        self._out_names = out_names
        self._out_avals = out_avals
        self._n_cores = n_cores
        self._jax = jax

    def submit(self):
        """Async-dispatch one execution; returns un-fetched device outputs."""
        return self._sharded(*self._dev_in, *self._dev_zeros)

    def __call__(self):
        outs = self.submit()
        # np.asarray blocks on the device result itself; an explicit
        # block_until_ready first would cost a second tunnel round trip.
        outs = [np.asarray(o) for o in outs]
        return [
            {nm: outs[i].reshape(self._n_cores, *self._out_avals[i].shape)[c]
             for i, nm in enumerate(self._out_names)}
            for c in range(self._n_cores)
        ]


def make_runner(inputs, n_steps=None):
    per_core = _prep_host(inputs)
    skip_b2 = bool(np.all(np.asarray(inputs["f_b2"]) == 0.0))
    zero_b01 = bool(np.all(np.asarray(inputs["f_b0"]) == 0.0)
                    and np.all(np.asarray(inputs["f_b1"]) == 0.0))
    nc = build_program(N_STEPS if n_steps is None else n_steps, skip_b2=skip_b2,
                       zero_b01=zero_b01)
    # codegen-level only (CoreSim can't ingest post-hoc instructions)
    _legalize_waits(nc)
    return Runner(nc, per_core)


def run(inputs):
    """Build + run on the 8 NeuronCores; returns output [256]."""
    runner = make_runner(inputs)
    results = runner()
    outs = [results[i]["out"].reshape(BL) for i in range(N_CORES)]
    return np.concatenate(outs).astype(np.float32)


def kernel(**inputs):
    return run(inputs)



